# revision 11
# baseline (speedup 1.0000x reference)
"""Trainium2 Bass kernel for nn_CALayer (FFT-magnitude channel attention).

Math per (b, c) image X [256, 256] (real):
  F(p, q) = 2D DFT;  y[b,c] = mean over the centered (fftshifted) 100x100
  low-frequency crop of |F|;  s = sigmoid(w2 @ relu(w1 @ y + b1) + b2);
  out = x * s[:, :, None, None].

Implementation: DFT-as-matmul with Hermitian reduction. Since X is real,
|F(-p,-q)| = |F(p,q)|, so only p in 0..50 (51 rows) and q in -50..50
(101 cols) of the spectrum are computed, and the crop sum over
p,q in [-50, 49]^2 is recovered as two separable window sums:
  S = sum_{q in -50..49} sum_{p in 0..49} |F| + sum_{q in -49..50} sum_{p in 1..50} |F|.

Dataflow per core (2 batches x 64 channels):
  - x[b] resident in SBUF as 2 h-chunks x 8 channel-groups of [128, 8, 256]
  - step A (PE): U^T[w, p] = X^T @ Wu   (image as stationary operand)
  - step B (PE): F^T[q, (ch, p)] = Wv^T @ U^T  (4 real matmuls per part)
  - mag (ACT/DVE): sqrt(Fr^2 + Fi^2)
  - crop sum: indicator matmul over q (PE) + windowed free-dim reduces (DVE)
    + tiny ones matmul accumulating y[1, 64] in PSUM
  - SE block on-device; s broadcast to all partitions via ones-matmul
  - in-place per-channel scale of the resident x tiles, DMA out

Sharding: pure data parallel over batch: core i handles batches 2i, 2i+1.
"""

import os
import sys

for _p in (
    "/root/.axon_site",
    "/root/.axon_site/_ro/trn_rl_repo",
    "/root/.axon_site/_ro/pypackages",
    "/opt/trn_rl_repo",
):
    if os.path.isdir(_p) and _p not in sys.path:
        sys.path.append(_p)

import numpy as np

import concourse.bacc as bacc
import concourse.mybir as mybir
import concourse.tile as tile
from concourse.bass_utils import run_bass_kernel_spmd

N_CORES = 8
B, C, H, W = 16, 64, 256, 256
BPC = B // N_CORES  # batches per core
CROP = 50
NP_ = 51   # p = 0..50
NQ = 101   # q = -50..50
GS = 8     # channels per group
NG = C // GS
F32 = mybir.dt.float32
F32R = mybir.dt.float32r
AF = mybir.ActivationFunctionType


# matmul precision mode: "f32" = plain fp32 (4 cyc/row, exact),
# "f32r_b" = fp32r for step B / crop-sum matmuls (1 cyc/row at N>=256,
# producers round on-chip), step A stays fp32.
MM_MODE = os.environ.get("CA_MM_MODE", "f32")


def _build_consts(w1, b1, w2, b2):
    h_idx = np.arange(H)
    ang_p = 2 * np.pi * np.outer(h_idx, np.arange(NP_)) / H
    wu = np.zeros((H, 102), np.float32)
    wu[:, :NP_] = np.cos(ang_p)
    wu[:, NP_:2 * NP_] = -np.sin(ang_p)
    ang_q = 2 * np.pi * np.outer(h_idx, np.arange(-CROP, CROP + 1)) / W
    cq = np.cos(ang_q).astype(np.float32)
    sq = np.sin(ang_q).astype(np.float32)
    wv = np.ascontiguousarray(np.concatenate([cq, sq, -sq], axis=1))  # [256, 303]
    r1 = np.zeros((NQ, 2), np.float32)
    r1[0:100, 0] = 1.0  # q in -50..49
    r1[1:101, 1] = 1.0  # q in -49..50
    return {
        "wu": wu,
        "wv": wv,
        "r1ind": r1,
        "id1": np.ones((1, 1), np.float32),
        "ones128": np.ones((1, 128), np.float32),
        "w1t": np.ascontiguousarray(w1.T.astype(np.float32) / 1e4),  # fold /10000
        "b1c": np.ascontiguousarray(b1.astype(np.float32).reshape(-1, 1)),
        "w2t": np.ascontiguousarray(w2.T.astype(np.float32)),
        "b2r": np.ascontiguousarray(b2.astype(np.float32).reshape(1, -1)),
    }


def _build_nc():
    nc = bacc.Bacc("TRN2", target_bir_lowering=False, debug=False)
    x_d = nc.dram_tensor("x", [BPC, C, H, W], F32, kind="ExternalInput").ap()
    out_d = nc.dram_tensor("out", [BPC, C, H, W], F32, kind="ExternalOutput").ap()
    wu_d = nc.dram_tensor("wu", [H, 102], F32, kind="ExternalInput").ap()
    wv_d = nc.dram_tensor("wv", [W, 303], F32, kind="ExternalInput").ap()
    r1_d = nc.dram_tensor("r1ind", [NQ, 2], F32, kind="ExternalInput").ap()
    id1_d = nc.dram_tensor("id1", [1, 1], F32, kind="ExternalInput").ap()
    ones128_d = nc.dram_tensor("ones128", [1, 128], F32, kind="ExternalInput").ap()
    w1t_d = nc.dram_tensor("w1t", [C, 4], F32, kind="ExternalInput").ap()
    b1c_d = nc.dram_tensor("b1c", [4, 1], F32, kind="ExternalInput").ap()
    w2t_d = nc.dram_tensor("w2t", [4, C], F32, kind="ExternalInput").ap()
    b2r_d = nc.dram_tensor("b2r", [1, C], F32, kind="ExternalInput").ap()

    with tile.TileContext(nc) as tc:
        with (
            tc.tile_pool(name="consts", bufs=1) as cpool,
            tc.tile_pool(name="xp", bufs=18) as xpool,
            tc.tile_pool(name="work", bufs=2) as wpool,
            tc.tile_pool(name="psA", bufs=3, space="PSUM") as pA,
            tc.tile_pool(name="psB", bufs=1, space="PSUM") as pB,
            tc.tile_pool(name="psS", bufs=1, space="PSUM") as pS,
        ):
            wu_sb = cpool.tile([128, 2, 102], F32, name="wu_sb")
            nc.sync.dma_start(wu_sb[:], wu_d.rearrange("(k p) n -> p k n", p=128))
            wv_sb = cpool.tile([128, 2, 303], F32, name="wv_sb")
            nc.sync.dma_start(wv_sb[:], wv_d.rearrange("(k p) n -> p k n", p=128))
            r1_sb = cpool.tile([NQ, 2], F32, name="r1_sb")
            nc.sync.dma_start(r1_sb[:], r1_d[:])
            id1_sb = cpool.tile([1, 1], F32, name="id1_sb")
            nc.sync.dma_start(id1_sb[:], id1_d[:])
            ones128_sb = cpool.tile([1, 128], F32, name="ones128_sb")
            nc.sync.dma_start(ones128_sb[:], ones128_d[:])
            w1t_sb = cpool.tile([C, 4], F32, name="w1t_sb")
            nc.sync.dma_start(w1t_sb[:], w1t_d[:])
            b1c_sb = cpool.tile([4, 1], F32, name="b1c_sb")
            nc.sync.dma_start(b1c_sb[:], b1c_d[:])
            w2t_sb = cpool.tile([4, C], F32, name="w2t_sb")
            nc.sync.dma_start(w2t_sb[:], w2t_d[:])
            b2r_sb = cpool.tile([1, C], F32, name="b2r_sb")
            nc.sync.dma_start(b2r_sb[:], b2r_d[:])

            stepB_r = MM_MODE == "f32r_b"
            mdt = F32R if stepB_r else F32
            if stepB_r:
                wv_mm = cpool.tile([128, 2, 303], F32R, name="wv_mm")
                nc.vector.tensor_copy(wv_mm[:], wv_sb[:])
                r1_mm = cpool.tile([NQ, 2], F32R, name="r1_mm")
                nc.vector.tensor_copy(r1_mm[:], r1_sb[:])
            else:
                wv_mm = wv_sb
                r1_mm = r1_sb

            for b in range(BPC):
                # ---- load the whole batch: 2 h-chunks x 8 groups of 8 channels
                xt = {}
                for g in range(NG):
                    for k in range(2):
                        t = xpool.tile([128, GS, W], F32, name="xt", tag="xt")
                        src = x_d[b].rearrange("c h w -> h c w")[
                            128 * k:128 * (k + 1), GS * g:GS * (g + 1), :
                        ]
                        nc.sync.dma_start(t[:], src)
                        xt[(k, g)] = t

                y_sb = wpool.tile([1, C], F32, name="y_sb", tag="y")

                for g in range(NG):
                    # ---- step A: U^T = X^T @ Wu per channel (both w-chunks)
                    u_sb = wpool.tile([128, GS * 204], mdt, name="u_sb", tag="u")
                    for j in range(GS):
                        for wk in range(2):
                            psA = pA.tile([128, 102], F32, name="psA", tag="uA")
                            for kk in range(2):
                                nc.tensor.matmul(
                                    psA[:],
                                    xt[(kk, g)][:, j, 128 * wk:128 * (wk + 1)],
                                    wu_sb[:, kk, :],
                                    start=(kk == 0),
                                    stop=(kk == 1),
                                )
                            dst = u_sb[:, j * 204 + 102 * wk: j * 204 + 102 * wk + 102]
                            if wk == 0:
                                nc.vector.tensor_copy(dst, psA[:])
                            else:
                                nc.scalar.copy(dst, psA[:])

                    # ---- step B: F^T[q, (ch, p)] with complex arithmetic
                    psB = pB.tile([NQ, 1024], F32, name="psB", tag="fB")
                    fr = psB[:, 0:408]
                    fi = psB[:, 512:920]
                    u3 = u_sb.rearrange("p (c x) -> p c x", c=GS)
                    fr_terms, fi_terms = [], []
                    for k in range(2):
                        ur = u3[:, :, 102 * k:102 * k + 51]
                        ui = u3[:, :, 102 * k + 51:102 * k + 102]
                        ck = wv_mm[:, k, 0:101]
                        sk = wv_mm[:, k, 101:202]
                        snk = wv_mm[:, k, 202:303]
                        fr_terms += [(ck, ur), (sk, ui)]
                        fi_terms += [(ck, ui), (snk, ur)]
                    for i, (lhsT, rhs) in enumerate(fr_terms):
                        nc.tensor.matmul(fr, lhsT, rhs, start=(i == 0), stop=(i == 3))
                    for i, (lhsT, rhs) in enumerate(fi_terms):
                        nc.tensor.matmul(fi, lhsT, rhs, start=(i == 0), stop=(i == 3))

                    # ---- |F| = sqrt(Fr^2 + Fi^2)
                    m2 = wpool.tile([NQ, 408], F32, name="m2", tag="m2")
                    m2b = wpool.tile([NQ, 408], F32, name="m2b", tag="m2b")
                    nc.scalar.square(m2[:], fr)
                    nc.scalar.square(m2b[:], fi)
                    nc.vector.tensor_add(m2[:], m2[:], m2b[:])
                    mag = wpool.tile([NQ, 408], mdt, name="mag", tag="mag")
                    nc.scalar.sqrt(mag[:], m2[:])

                    # ---- crop sum: two q-window matmuls (windowed over p in the
                    # rhs AP), p reduces from PSUM, then per-group y row slice
                    mag3 = mag.rearrange("p (c x) -> p c x", c=GS)
                    g2_ps = pS.tile([1, 1024], F32, name="g2_ps", tag="G")
                    nc.tensor.matmul(
                        g2_ps[0:1, 0:400], r1_mm[:, 0:1], mag3[:, :, 0:50],
                        start=True, stop=True,
                    )
                    nc.tensor.matmul(
                        g2_ps[0:1, 512:912], r1_mm[:, 1:2], mag3[:, :, 1:51],
                        start=True, stop=True,
                    )
                    gred = wpool.tile([2, GS], F32, name="gred", tag="gred")
                    ga = g2_ps[0:1, 0:400].rearrange("p (c x) -> p c x", c=GS)
                    gb = g2_ps[0:1, 512:912].rearrange("p (c x) -> p c x", c=GS)
                    nc.vector.reduce_sum(gred[0:1, :], ga, axis=mybir.AxisListType.X)
                    gredb = wpool.tile([1, GS], F32, name="gredb", tag="gredb")
                    nc.vector.reduce_sum(gredb[0:1, :], gb, axis=mybir.AxisListType.X)
                    nc.vector.tensor_add(
                        y_sb[0:1, GS * g:GS * (g + 1)], gred[0:1, :], gredb[0:1, :]
                    )

                # ---- SE block (y is pre-divided by 1e4 via w1t folding)
                yT_ps = pS.tile([C, 1], F32, name="yT_ps", tag="se")
                nc.tensor.transpose(yT_ps[:], y_sb[:], id1_sb[:])
                y_col = wpool.tile([C, 1], F32, name="y_col", tag="se2")
                nc.scalar.copy(y_col[:], yT_ps[:])
                h_ps = pS.tile([4, 1], F32, name="h_ps", tag="se")
                nc.tensor.matmul(h_ps[:], w1t_sb[:], y_col[:], start=True, stop=True)
                h_sb = wpool.tile([4, 1], F32, name="h_sb", tag="se3")
                nc.scalar.activation(h_sb[:], h_ps[:], AF.Relu, bias=b1c_sb[:])
                sarg_ps = pS.tile([1, C], F32, name="sarg_ps", tag="se")
                nc.tensor.matmul(sarg_ps[:], h_sb[:], w2t_sb[:], start=True, stop=True)
                sarg_sb = wpool.tile([1, C], F32, name="sarg_sb", tag="se4")
                nc.vector.tensor_add(sarg_sb[:], sarg_ps[:], b2r_sb[:])
                s_row = wpool.tile([1, C], F32, name="s_row", tag="se5")
                nc.scalar.activation(s_row[:], sarg_sb[:], AF.Sigmoid)
                sb_ps = pS.tile([128, C], F32, name="sb_ps", tag="se")
                nc.tensor.matmul(
                    sb_ps[:], ones128_sb[:], s_row[:], start=True, stop=True
                )
                s_b = wpool.tile([128, C], F32, name="s_b", tag="se6")
                nc.vector.tensor_copy(s_b[:], sb_ps[:])

                # ---- in-place scale + writeback
                idx = 0
                for k in range(2):
                    for g in range(NG):
                        t = xt[(k, g)]
                        for j in range(GS):
                            sc = s_b[:, GS * g + j:GS * g + j + 1]
                            sl = t[:, j, :]
                            if idx % 3 == 2:
                                nc.scalar.mul(sl, sl, sc)
                            else:
                                nc.vector.tensor_scalar_mul(sl, sl, sc)
                            idx += 1
                        dst = out_d[b].rearrange("c h w -> h c w")[
                            128 * k:128 * (k + 1), GS * g:GS * (g + 1), :
                        ]
                        nc.sync.dma_start(dst, t[:])

    nc.compile()
    return nc


_NC = None


def _get_nc():
    global _NC
    if _NC is None:
        _NC = _build_nc()
    return _NC


def _execute(inputs, trace=False):
    x = np.ascontiguousarray(np.asarray(inputs["x"], dtype=np.float32))
    consts = _build_consts(
        np.asarray(inputs["w1"]), np.asarray(inputs["b1"]),
        np.asarray(inputs["w2"]), np.asarray(inputs["b2"]),
    )
    in_maps = []
    for i in range(N_CORES):
        m = {"x": np.ascontiguousarray(x[BPC * i:BPC * (i + 1)])}
        m.update(consts)
        in_maps.append(m)
    nc = _get_nc()
    res = run_bass_kernel_spmd(nc, in_maps, core_ids=list(range(N_CORES)), trace=trace)
    out = np.concatenate([res.results[i]["out"] for i in range(N_CORES)], axis=0)
    return out, res


def kernel(x, w1, b1, w2, b2):
    out, _ = _execute({"x": x, "w1": w1, "b1": b1, "w2": w2, "b2": b2}, trace=False)
    return out
